# revision 24
# baseline (speedup 1.0000x reference)
"""Trainium2 Bass kernel for BatchTemporalContrastiveLoss.

Strategy (pure data-parallel over 8 NeuronCores, batch B=16384 -> 2048/core):
  - Host: cast inputs to fp8(e4m3) matmul layouts with DoubleRow k-pairing
    (two 128-row k-subtiles per instruction -> 0.5 cyc/row on PE), build a
    per-row multiplicity histogram W[b, j] of neg_indices (the on-device
    "gather" becomes a dense weighted reduction), shard batch across cores.
  - Device (per core), all matmuls fp8 DoubleRow with fp32 PSUM:
      L1:  Y1^T = relu(W1s^T @ X^T) / 32          (W1 host-scaled by 32)
      L2:  zg^T = W2c^T @ Y1^T  computed TRANSPOSED [p, b] directly, where
           W2c is host column-centered (folds the LayerNorm mean away) and
           host-scaled; with beta=0, g=1, b2=0, b1=0 every scalar scale
           cancels against the L2 norm, so zg only ever needs per-row
           1/||zg|| corrections applied as activation scales.
      zgT is cast to fp8 (anchT/posT); per-row dots and squared norms are
      diagonals of 128x128 PE grams, extracted 4-at-a-time by masking with
      a block identity and a 3D tensor_reduce (DVE).
      S = anchT^T @ negT (PE);  E = exp(S * 2/||zg_a||) (ACT);
      negsum = sum_j W*E: the multiply is split DVE/Pool halves, then a
      binary add-tree at DVE 2x rate finishes with one reduce_sum.
      loss_b = ln(exp(pos2) + negsum) - pos2, batched [128,4] per slab.
  - Host: mean of the 16384 per-row losses (float64) -> scalar fp32.
"""

import os
import sys

import numpy as np
import ml_dtypes

_TRN_REPO = "/opt/trn_rl_repo"
if _TRN_REPO not in sys.path:
    sys.path.insert(0, _TRN_REPO)

import concourse.bass as bass
import concourse.bacc as bacc
import concourse.tile as tile
from concourse import mybir
from concourse.bass_utils import run_bass_kernel_spmd

# Steer bacc's activation-table chooser to the one set that contains all of
# relu/copy/exp/ln (it greedily picks the first set containing each function,
# causing ~1.3us table swaps between Exp and Ln otherwise).  The fast path
# only ever uses relu/copy/exp/ln, all present in natural_log_exp_and_others,
# so keeping just that set means a single boot-time table load.  The general
# path additionally needs sqrt, so it keeps sqrt_and_others too.
_ACT_KEEP = ["natural_log_exp_and_others"]
if not getattr(bacc.get_activation_tables, "_combined", False):
    _orig_get_act_tables = bacc.get_activation_tables

    def _combined_act_tables(arch):
        tables = _orig_get_act_tables(arch)
        return {name: (funcs if name in _ACT_KEEP else set())
                for name, funcs in tables.items()}

    _combined_act_tables._combined = True
    bacc.get_activation_tables = _combined_act_tables

F32 = mybir.dt.float32
BF16 = mybir.dt.bfloat16
FP8 = mybir.dt.float8e4
AF = mybir.ActivationFunctionType
ALU = mybir.AluOpType
PM = mybir.MatmulPerfMode.DoubleRow
BF = ml_dtypes.bfloat16
E4M3 = ml_dtypes.float8_e4m3

# Problem constants (hardcoded per spec)
B, H, HH, P, K, NBUF = 16384, 2048, 1024, 256, 64, 2000
NCORES = 8
BL = B // NCORES          # 2048 rows per core
JP = 2048                 # negatives padded to 2048 columns
BT = BL // 128            # 16 anchor b-tiles of 128 rows per core
W1S = 32.0                # host scale on W1 (fp8 range); relu divides by 32
W2S = 32.0                # host scale on W2c (cancels in normalization)
LN2 = 0.6931471805599453

LAST_RESULTS = None       # BassKernelResults of the most recent run
_NC_CACHE = {}

import os as _os
CFG = {
    'l1b': int(_os.environ.get('K_L1B', 2)),
    'zg': _os.environ.get('K_ZG', 'split'),   # 'split' 2x[128,512] | 'pair' 1x[128,1024]
    'spb': int(_os.environ.get('K_SPB', 3)),
    'gram': _os.environ.get('K_GRAM', 'gps'),  # 'gps' | 'zg' | 'sp'
    'pmult': int(_os.environ.get('K_PMULT', 0)),  # negsum mults on Pool out of 4
    'reluact': tuple(int(x) for x in
                     _os.environ.get('K_RELUACT', '3,5,7').split(',')),
    # negsum reduce engine per tile tt: 'd'=DVE reduce_sum, 'a'=ACT accum,
    # 's'=split halves across both
    'nred': _os.environ.get('K_NRED', 'tttt'),
    # gram-diag reduce engine, cycled: 'd'=DVE, 'a'=ACT
    'dred': _os.environ.get('K_DRED', 'dd'),
    # last pair's negatives: 'late' after pos_post, 'early' interleaved
    'tail3': _os.environ.get('K_TAIL3', 'late'),
    # negative scheduling: 'defer4' = all 4 tiles one pair later;
    # 'split22' = tiles (0,1) inline in own pair, (2,3) one pair later
    'negsched': _os.environ.get('K_NEGSCHED', 'defer4'),
    # zgT->fp8 cast engines: 'mix' = ACT c0 / DVE c1, 'act' = both on ACT
    'cast': _os.environ.get('K_CAST', 'act'),
    # reduce mode string used for the last pair's tiles (ACT idles there)
    'tailnred': _os.environ.get('K_TAILNRED', 'ttss'),
    # interleave prev-pair S-matmuls inside anchor-slab L1 (ring relief)
    'l1pad': int(_os.environ.get('K_L1PAD', 0)),
    # pair 0 computes its tiles (0,1) inline (no prior pair to pad with)
    'p0inline': int(_os.environ.get('K_P0INLINE', 0)),
    # last pair: emit S-sims before pos_post so the longer negsum chain
    # starts earlier in the tail
    'tailswap': int(_os.environ.get('K_TAILSWAP', 0)),
    # boot relu ACT-groups (boot relus all fire at once when the last DMA
    # lands; alternating engines frees the early l1ps slots sooner)
    'bootact': tuple(int(x) for x in
                     _os.environ.get('K_BOOTACT', '3,5,7').split(',')),
}

# v2 = anchors-first schedule (_emit_fast2); v1 = paired schedule
V2 = _os.environ.get('K_V2', '0') == '1'


def _parse_sched(s):
    """'0,1|2,3||4' -> [[0,1],[2,3],[],[4]]"""
    return [[int(x) for x in part.split(',') if x != '']
            for part in s.split('|')]


CFG2 = {
    # exps for tile t emitted in slab exp_sched[s] (sims+exp woven into L1)
    'exps': _os.environ.get(
        'K2_EXPS', '0,1|2,3|4,5|6,7|8,9|10,11|12,13|14,15'),
    # E*W trees per slab; remainder runs in the tail after slab 7
    'trees': _os.environ.get(
        'K2_TREES', '||0,1|2,3|4,5|6,7|8,9,10|11,12,13'),
    # tree level engines: m0 m1 s1 s2 s3 s4 red  (d=DVE, p=Pool; red is DVE)
    'tree': _os.environ.get('K2_TREE', 'dpddddd'),
    # odd tiles can use a different engine map (balance DVE vs Pool)
    'tree2': _os.environ.get('K2_TREE2', ''),
    'treetail': _os.environ.get('K2_TREETAIL', 'dpddddd'),
    # diag4 reduce: d=DVE reduce; p=Pool pre-tree (3 levels) + DVE finish
    'dred': _os.environ.get('K2_DRED', 'd'),
    # relu groups on ACT (of 8 per slab); others on DVE
    'reluact': tuple(int(x) for x in
                     _os.environ.get('K2_RELUACT', '3,5,7').split(',')),
    'bootact': tuple(int(x) for x in
                     _os.environ.get('K2_BOOTACT', '3,5,7').split(',')),
    # zg cast engines for the two 512 chunks: a=ACT, d=DVE
    'cast': _os.environ.get('K2_CAST', 'aa'),
    # L1 pad positions (after relu of these groups) for woven work items
    'padpos': tuple(int(x) for x in
                    _os.environ.get('K2_PADPOS', '1,2,3,4,5,6,7').split(',')),
    'eb': int(_os.environ.get('K2_EB', 6)),
}


def _emit_fast2(tc, out_losses, ins, ident_dram):
    """Anchors-first fp8 fast path.

    Slab order s=0..7 over 512-row blocks: [A0 A1 A2 A3 P0 P1 P2 P3].
    All four anchor projections complete by mid-kernel, so the negative
    S-matmul/exp/weighted-tree work for all 16 anchor b-tiles spreads
    across the whole kernel instead of piling into a tail.  Per-tile
    work items (sims+exp on PE/ACT, tree pieces on DVE/Pool) are woven
    into the L1 matmul stream of each slab at relu-group boundaries.

    PSUM: l1 2x[128,512] + zg 2x[128,512] + sp 2x[128,8,128] = 8 banks.
    S-chunks pair into one 2-bank psum tile -> one exp per 1024 cols;
    gram tiles share the sp pool (no dedicated gram bank).  DMA is
    need-ordered: x slabs split in halves, wcnt split per tile-pair.
    """
    from contextlib import ExitStack

    nc = tc.nc
    C = CFG2
    exp_sched = _parse_sched(C['exps'])
    tree_sched = _parse_sched(C['trees'])
    assert len(exp_sched) == 8 and len(tree_sched) == 8
    all_tiles = sorted(t for part in exp_sched for t in part)
    assert all_tiles == list(range(BT)), exp_sched
    tree_tiles = [t for part in tree_sched for t in part]
    tail_trees = [t for t in range(BT) if t not in tree_tiles]
    exp_slab = {t: s for s, part in enumerate(exp_sched) for t in part}
    for s, part in enumerate(tree_sched):
        for t in part:
            assert exp_slab[t] <= s, (t, s)

    with ExitStack() as ctx:
        const = ctx.enter_context(tc.tile_pool(name="const", bufs=1))

        w1s = [const.tile([128, 2, 1024], FP8, tag=f"w1s{kk}",
                          name=f"w1s{kk}") for kk in range(8)]
        w2s = const.tile([128, 4, 2, 256], FP8, tag="w2s")
        negtp = const.tile([128, 2, JP], FP8, tag="negtp")
        identb = const.tile([128, 4, 128], F32, tag="identb")
        ln2c = const.tile([128, 1], F32, tag="ln2c")
        nc.gpsimd.memset(ln2c[:], LN2)
        # identb[p, a, c] = (c == p): built on Pool at boot, off the DMA track
        nc.gpsimd.memset(identb[:], 1.0)
        nc.gpsimd.affine_select(
            identb[:], identb[:], pattern=[[0, 4], [1, 128]],
            compare_op=ALU.is_equal, fill=0.0, base=0,
            channel_multiplier=-1)

        nsqA = const.tile([128, BT], F32, tag="nsqA")
        nsqP = const.tile([128, BT], F32, tag="nsqP")
        prawT = const.tile([128, BT], F32, tag="prawT")
        rn2a = const.tile([128, BT], F32, tag="rn2a")
        rpmT = const.tile([128, BT], F32, tag="rpmT")
        pos2T = const.tile([128, BT], F32, tag="pos2T")
        negsumS = const.tile([128, BT], F32, tag="negsumS")
        lossT = const.tile([128, BT], F32, tag="lossT")

        xbpool = ctx.enter_context(tc.tile_pool(name="xb", bufs=8))
        xpool = ctx.enter_context(tc.tile_pool(
            name="xk", bufs=int(_os.environ.get('K2_XB', 10))))
        y1pool = ctx.enter_context(tc.tile_pool(name="y1", bufs=8))
        atpool = ctx.enter_context(tc.tile_pool(name="at", bufs=4))
        ptpool = ctx.enter_context(tc.tile_pool(name="pt", bufs=2))
        epool = ctx.enter_context(tc.tile_pool(name="ep", bufs=C['eb']))
        wpool = ctx.enter_context(tc.tile_pool(
            name="wp", bufs=int(_os.environ.get('K2_WB', 4))))
        dpool = ctx.enter_context(tc.tile_pool(
            name="dp", bufs=int(_os.environ.get('K2_DB', 4))))
        smp = ctx.enter_context(tc.tile_pool(name="small", bufs=6))

        # ---- boot: slab 0 (A0) with kk-outer over 8 open PSUM groups ----
        xk0 = []
        for kk in range(8):
            nc.sync.dma_start(w1s[kk][:], ins["w1"][:, kk, :, :])
            xt_t = xbpool.tile([128, 2, 512], FP8, tag="xboot",
                               name=f"xb{kk}")
            nc.sync.dma_start(xt_t[:], ins["xt"][:, 0, kk, :, :])
            xk0.append(xt_t)

        l1ps = ctx.enter_context(tc.tile_pool(
            name="l1ps", bufs=int(_os.environ.get('K2_L1B', 4)),
            space="PSUM"))
        sps = ctx.enter_context(tc.tile_pool(
            name="sps", bufs=int(_os.environ.get('K2_SPB', 2)),
            space="PSUM"))

        y1_slab0 = []
        spE = sps.tile([128, 8, 128], F32, tag="sp", name="bpEF")
        spG = sps.tile([128, 8, 128], F32, tag="sp", name="bpGH")
        ps0 = [l1ps.tile([128, 4, 128], F32, tag="l1", name="bpA")[:],
               l1ps.tile([128, 4, 128], F32, tag="l1", name="bpB")[:],
               l1ps.tile([128, 4, 128], F32, tag="l1", name="bpC")[:],
               l1ps.tile([128, 4, 128], F32, tag="l1", name="bpD")[:],
               spE[:, 0:4, :], spE[:, 4:8, :],
               spG[:, 0:4, :], spG[:, 4:8, :]]
        for kk in range(8):
            for n1 in range(8):
                nc.tensor.matmul(
                    ps0[n1],
                    w1s[kk][:, :, n1 * 128:(n1 + 1) * 128],
                    xk0[kk][:, :, :],
                    start=(kk == 0), stop=(kk == 7), perf_mode=PM,
                )
        for n1 in range(8):
            kk2, i = n1 // 2, n1 % 2
            if i == 0:
                y1_t = y1pool.tile([128, 2, 512], FP8, tag="y1",
                                   name=f"y1b{kk2}")
                y1_slab0.append(y1_t)
            if n1 in C['bootact']:
                nc.scalar.activation(y1_slab0[kk2][:, i, :], ps0[n1], AF.Relu,
                                     bias=0.0, scale=1.0 / 32.0)
            else:
                nc.vector.tensor_scalar(y1_slab0[kk2][:, i, :], ps0[n1], 0.0,
                                        1.0 / 32.0, ALU.max, ALU.mult)

        def dma_x(s):
            """x slab s in two kk-halves so L1 can start at half-landing."""
            xa = xpool.tile([128, 4, 2, 512], FP8, tag="xs", name=f"xs{s}a")
            nc.sync.dma_start(xa[:], ins["xt"][:, s, 0:4, :, :])
            xb = xpool.tile([128, 4, 2, 512], FP8, tag="xs", name=f"xs{s}b")
            nc.sync.dma_start(xb[:], ins["xt"][:, s, 4:8, :, :])
            return (xa, xb)

        def dma_wc(a, h):
            """wcnt for pair a, tile half h (tiles 4a+2h, 4a+2h+1)."""
            wcs = wpool.tile([128, 2, JP], BF16, tag="wc", name=f"wc{a}{h}")
            nc.sync.dma_start(wcs[:], ins["wcnt"][:, a, 2 * h:2 * h + 2, :])
            wcs_tiles[(a, h)] = wcs

        def emit_relu(out_ap, ps_ap, n1):
            if n1 in C['reluact']:
                nc.scalar.activation(out_ap, ps_ap, AF.Relu, bias=0.0,
                                     scale=1.0 / 32.0)
            else:
                nc.vector.tensor_scalar(out_ap, ps_ap, 0.0, 1.0 / 32.0,
                                        ALU.max, ALU.mult)

        def emit_l1(s, xs2, work):
            """L1 for slab s; `work` is a list of zero-arg callbacks woven
            in after the relus of groups C['padpos']."""
            pads = {}
            if work:
                pp = C['padpos']
                for i, cb in enumerate(work):
                    pos = pp[min(i * len(pp) // len(work), len(pp) - 1)]
                    pads.setdefault(pos, []).append(cb)
            y1 = []
            for n1 in range(8):
                kk2, i = n1 // 2, n1 % 2
                if i == 0:
                    y1_t = y1pool.tile([128, 2, 512], FP8, tag="y1",
                                       name=f"y1_{s}_{kk2}")
                    y1.append(y1_t)
                ps = l1ps.tile([128, 4, 128], F32, tag="l1")
                for kk in range(8):
                    nc.tensor.matmul(
                        ps[:, :, :],
                        w1s[kk][:, :, n1 * 128:(n1 + 1) * 128],
                        xs2[kk // 4][:, kk % 4, :, :],
                        start=(kk == 0), stop=(kk == 7), perf_mode=PM,
                    )
                emit_relu(y1[kk2][:, i, :], ps[:, :, :], n1)
                for cb in pads.get(n1, ()):
                    cb()
            return y1

        def emit_l2(s, y1, tpool):
            zt = tpool.tile([128, 2, 512], FP8, tag="zt", name=f"zt{s}")
            zp = sps.tile([128, 8, 128], F32, tag="sp", name=f"zg{s}")
            pss = [zp[:, 0:4, :], zp[:, 4:8, :]]
            for kk2 in range(4):
                for c in range(2):
                    nc.tensor.matmul(
                        pss[c],
                        w2s[:, kk2, :, c * 128:(c + 1) * 128],
                        y1[kk2][:, :, :],
                        start=(kk2 == 0), stop=(kk2 == 3), perf_mode=PM,
                    )
            for c in range(2):
                if C['cast'][c] == 'a':
                    nc.scalar.activation(zt[:, c, :], pss[c], AF.Copy)
                else:
                    nc.vector.tensor_copy(zt[:, c, :], pss[c])
            return zt

        def emit_diag4(gt_ap, out_cols):
            gd = dpool.tile([128, 4, 128], F32, tag="gd")
            nc.vector.tensor_tensor(gd[:], gt_ap, identb[:], ALU.mult)
            if C['dred'] == 'p':
                # Pool folds 128 -> 64 once, DVE finishes (gpsimd cannot
                # reduce along the free axis)
                g2 = dpool.tile([128, 4, 64], F32, tag="gd2")
                nc.gpsimd.tensor_tensor(g2[:], gd[:, :, 0:64],
                                        gd[:, :, 64:128], ALU.add)
                nc.vector.tensor_reduce(out_cols, g2[:],
                                        mybir.AxisListType.X, ALU.add)
            else:
                nc.vector.tensor_reduce(out_cols, gd[:],
                                        mybir.AxisListType.X, ALU.add)

        def emit_anchor_post(a, anchT):
            gt = l1ps.tile([128, 4, 128], F32, tag="l1", name=f"gA{a}")
            for tt in range(4):
                at = anchT[:, :, tt * 128:(tt + 1) * 128]
                nc.tensor.matmul(gt[:, tt, :], at, at, start=True, stop=True,
                                 perf_mode=PM)
            cols = slice(4 * a, 4 * a + 4)
            emit_diag4(gt[:, :, :], nsqA[:, cols])
            lnt = smp.tile([128, 4], F32, tag="lnt")
            nc.scalar.activation(lnt[:], nsqA[:, cols], AF.Ln)
            nc.scalar.activation(rn2a[:, cols], lnt[:], AF.Exp,
                                 bias=ln2c[:, 0:1], scale=-0.5)

        anchTs = {}
        edict = {}
        tstate = {}
        wcs_tiles = {}

        def emit_sims_exp(t):
            a, tt = t // 4, t % 4
            at = anchTs[a][:, :, tt * 128:(tt + 1) * 128]
            E = epool.tile([128, JP], BF16, tag="E", name=f"E{t}")
            for p in range(2):
                ps = sps.tile([128, 8, 128], F32, tag="sp")
                for h in range(2):
                    jc = 2 * p + h
                    nc.tensor.matmul(
                        ps[:, 4 * h:4 * h + 4, :], at,
                        negtp[:, :, jc * 512:(jc + 1) * 512],
                        start=True, stop=True, perf_mode=PM)
                nc.scalar.activation(E[:, p * 1024:(p + 1) * 1024],
                                     ps[:, :, :], AF.Exp, bias=0.0,
                                     scale=rn2a[:, t:t + 1])
            edict[t] = E

        def _tt(eng, out, a, b, op):
            if eng == 'p':
                nc.gpsimd.tensor_tensor(out, a, b, op)
            else:
                nc.vector.tensor_tensor(out, a, b, op)

        def _tree_eng(t, tail):
            if tail:
                return C['treetail']
            if C['tree2'] and t % 2 == 1:
                return C['tree2']
            return C['tree']

        def emit_tree_a(t, tail=False):
            """mults + first add (phases m0/m1/s1)."""
            eng = _tree_eng(t, tail)
            E = edict.pop(t)
            wcs = wcs_tiles[(t // 4, (t % 4) // 2)]
            ti = t % 2
            H = JP // 2
            m0 = dpool.tile([128, H], BF16, tag="m0")
            m1 = dpool.tile([128, H], BF16, tag="m1")
            _tt(eng[0], m0[:], E[:, 0:H], wcs[:, ti, 0:H], ALU.mult)
            _tt(eng[1], m1[:], E[:, H:JP], wcs[:, ti, H:JP], ALU.mult)
            s1 = dpool.tile([128, H], BF16, tag="s1")
            _tt(eng[2], s1[:], m0[:], m1[:], ALU.add)
            tstate[t] = s1

        def emit_tree_b(t, tail=False):
            """rest of the add tree + final reduce."""
            eng = _tree_eng(t, tail)
            s1 = tstate.pop(t)
            s2 = smp.tile([128, 512], BF16, tag="s2")
            _tt(eng[3], s2[:], s1[:, 0:512], s1[:, 512:1024], ALU.add)
            s3 = smp.tile([128, 256], BF16, tag="s3")
            _tt(eng[4], s3[:], s2[:, 0:256], s2[:, 256:512], ALU.add)
            s4 = smp.tile([128, 128], BF16, tag="s4")
            _tt(eng[5], s4[:], s3[:, 0:128], s3[:, 128:256], ALU.add)
            nc.vector.reduce_sum(negsumS[:, t:t + 1], s4[:],
                                 axis=mybir.AxisListType.X)

        def emit_pos_post(k, anchT, posT):
            cols = slice(4 * k, 4 * k + 4)
            gtA = l1ps.tile([128, 4, 128], F32, tag="l1", name=f"gPa{k}")
            for tt in range(4):
                at = anchT[:, :, tt * 128:(tt + 1) * 128]
                pt = posT[:, :, tt * 128:(tt + 1) * 128]
                nc.tensor.matmul(gtA[:, tt, :], at, pt,
                                 start=True, stop=True, perf_mode=PM)
            gtB = l1ps.tile([128, 4, 128], F32, tag="l1", name=f"gPb{k}")
            for tt in range(4):
                pt = posT[:, :, tt * 128:(tt + 1) * 128]
                nc.tensor.matmul(gtB[:, tt, :], pt, pt,
                                 start=True, stop=True, perf_mode=PM)
            emit_diag4(gtA[:, :, :], prawT[:, cols])
            emit_diag4(gtB[:, :, :], nsqP[:, cols])
            lnt = smp.tile([128, 4], F32, tag="lnt")
            nc.scalar.activation(lnt[:], nsqP[:, cols], AF.Ln)
            nc.scalar.activation(rpmT[:, cols], lnt[:], AF.Exp, bias=0.0,
                                 scale=-0.5)
            pr = smp.tile([128, 4], F32, tag="pr")
            nc.vector.tensor_tensor(pr[:], prawT[:, cols], rn2a[:, cols],
                                    ALU.mult)
            nc.vector.tensor_tensor(pos2T[:, cols], pr[:], rpmT[:, cols],
                                    ALU.mult)

        def emit_loss(k):
            cols = slice(4 * k, 4 * k + 4)
            pe = smp.tile([128, 4], F32, tag="pe")
            nc.scalar.activation(pe[:], pos2T[:, cols], AF.Exp)
            tot = smp.tile([128, 4], F32, tag="tot")
            nc.vector.tensor_tensor(tot[:], pe[:], negsumS[:, cols], ALU.add)
            lse = smp.tile([128, 4], F32, tag="lse")
            nc.scalar.activation(lse[:], tot[:], AF.Ln)
            nc.vector.tensor_tensor(lossT[:, cols], lse[:], pos2T[:, cols],
                                    ALU.subtract)

        last_tree_slab = {}
        for s, part in enumerate(tree_sched):
            for t in part:
                k = t // 4
                last_tree_slab[k] = max(last_tree_slab.get(k, 0), s)
        for t in tail_trees:
            last_tree_slab[t // 4] = 99
        loss_slab = {k: max(last_tree_slab[k], 4 + k) for k in range(4)}

        # ---- steady state; DMA queue is strictly need-ordered ----
        nc.sync.dma_start(w2s[:], ins["w2"][:, :, :, :])
        xs_tiles = {1: dma_x(1)}
        nc.sync.dma_start(negtp[:], ins["negt"][:, :, :])

        anchTs[0] = emit_l2(0, y1_slab0, atpool)
        emit_anchor_post(0, anchTs[0])
        for t in exp_sched[0]:
            emit_sims_exp(t)
        xs_tiles[2] = dma_x(2)
        dma_wc(0, 0)

        # slab -> list of DMA thunks, in queue order
        dma_plan = {
            1: [lambda: xs_tiles.update({3: dma_x(3)}), lambda: dma_wc(0, 1)],
            2: [lambda: dma_wc(1, 0), lambda: dma_wc(1, 1)],
            3: [lambda: xs_tiles.update({4: dma_x(4)}),
                lambda: dma_wc(2, 0)],
            4: [lambda: xs_tiles.update({5: dma_x(5)}),
                lambda: dma_wc(2, 1)],
            5: [lambda: xs_tiles.update({6: dma_x(6)}),
                lambda: dma_wc(3, 0), lambda: dma_wc(3, 1)],
            6: [lambda: xs_tiles.update({7: dma_x(7)})],
        }

        for s in range(1, 8):
            pair = s if s < 4 else s - 4
            work = []
            for t in exp_sched[s]:
                work.append(lambda t=t: emit_sims_exp(t))
            for t in tree_sched[s]:
                work.append(lambda t=t: emit_tree_a(t))
                work.append(lambda t=t: emit_tree_b(t))
            y1 = emit_l1(s, xs_tiles[s], work)
            for thunk in dma_plan.get(s, ()):
                thunk()
            if s < 4:
                anchTs[s] = emit_l2(s, y1, atpool)
                emit_anchor_post(s, anchTs[s])
            else:
                posT = emit_l2(s, y1, ptpool)
                emit_pos_post(pair, anchTs[pair], posT)
            for k in range(4):
                if loss_slab[k] == s:
                    emit_loss(k)

        # ---- tail ----
        for t in tail_trees:
            emit_tree_a(t, tail=True)
            emit_tree_b(t, tail=True)
        for k in range(4):
            if loss_slab[k] > 7:
                emit_loss(k)

        nc.sync.dma_start(out_losses[:, :], lossT[:])


def _emit_fast(tc, out_losses, ins, ident_dram):
    """fp8 DoubleRow fast path (b1==0, b2==0, gamma==1, beta==0).

    Slab order s=0..7 over 512-row blocks: [A0 P0 A1 P1 A2 P2 A3 P3].
    Even s = anchor slab a=s//2 (tiles 4a..4a+3), odd s = positive slab.
    """
    from contextlib import ExitStack

    nc = tc.nc
    with ExitStack() as ctx:
        const = ctx.enter_context(tc.tile_pool(name="const", bufs=1))

        w1s = [const.tile([128, 2, 1024], FP8, tag=f"w1s{kk}",
                          name=f"w1s{kk}") for kk in range(8)]
        w2s = const.tile([128, 4, 2, 256], FP8, tag="w2s")
        negtp = const.tile([128, 2, JP], FP8, tag="negtp")
        identb = const.tile([128, 4, 128], F32, tag="identb")
        ln2c = const.tile([128, 1], F32, tag="ln2c")
        nc.gpsimd.memset(ln2c[:], LN2)

        nsqA = const.tile([128, BT], F32, tag="nsqA")
        nsqP = const.tile([128, BT], F32, tag="nsqP")
        prawT = const.tile([128, BT], F32, tag="prawT")
        rn2a = const.tile([128, BT], F32, tag="rn2a")
        rpmT = const.tile([128, BT], F32, tag="rpmT")
        pos2T = const.tile([128, BT], F32, tag="pos2T")
        negsumS = const.tile([128, BT], F32, tag="negsumS")
        lossT = const.tile([128, BT], F32, tag="lossT")

        xbpool = ctx.enter_context(tc.tile_pool(name="xb", bufs=8))
        xpool = ctx.enter_context(tc.tile_pool(name="xk", bufs=3))
        y1pool = ctx.enter_context(tc.tile_pool(name="y1", bufs=8))
        atpool = ctx.enter_context(tc.tile_pool(name="at", bufs=2))
        ptpool = ctx.enter_context(tc.tile_pool(name="pt", bufs=2))
        epool = ctx.enter_context(tc.tile_pool(name="ep", bufs=6))
        wpool = ctx.enter_context(tc.tile_pool(name="wp", bufs=2))
        dpool = ctx.enter_context(tc.tile_pool(
            name="dp", bufs=int(_os.environ.get('K2_DB', 4))))
        bpool = ctx.enter_context(tc.tile_pool(name="bp", bufs=2))
        smp = ctx.enter_context(tc.tile_pool(name="small", bufs=4))

        # ---- boot: slab 0 with kk-outer over 8 open PSUM groups, so PE
        # saturates as soon as the first w1/x chunk pair lands ----
        xk0 = []
        for kk in range(8):
            nc.sync.dma_start(w1s[kk][:], ins["w1"][:, kk, :, :])
            xt_t = xbpool.tile([128, 2, 512], FP8, tag="xboot",
                               name=f"xb{kk}")
            nc.sync.dma_start(xt_t[:], ins["xt"][:, 0, kk, :, :])
            xk0.append(xt_t)

        l1ps = ctx.enter_context(tc.tile_pool(name="l1ps", bufs=CFG['l1b'],
                                              space="PSUM"))
        zgps = ctx.enter_context(tc.tile_pool(
            name="zgps", bufs=(1 if CFG['zg'] == 'pair' else 2), space="PSUM"))
        sps = ctx.enter_context(tc.tile_pool(name="sps", bufs=CFG['spb'],
                                             space="PSUM"))
        if CFG['gram'] == 'gps':
            gps = ctx.enter_context(tc.tile_pool(name="gps", bufs=1,
                                                 space="PSUM"))

        # boot psum groups borrow the steady pools' banks (a dedicated boot
        # pool would insert a full drain barrier on close)
        y1_slab0 = []
        ps0 = [l1ps.tile([128, 512], F32, tag="l1", name="bpA"),
               l1ps.tile([128, 512], F32, tag="l1", name="bpB"),
               zgps.tile([128, 512], F32, tag="zg", name="bpC"),
               zgps.tile([128, 512], F32, tag="zg", name="bpD"),
               sps.tile([128, 512], F32, tag="sp", name="bpE"),
               sps.tile([128, 512], F32, tag="sp", name="bpF"),
               sps.tile([128, 512], F32, tag="sp", name="bpG"),
               gps.tile([128, 4, 128], F32, tag="g", name="bpH")]
        for kk in range(8):
            for n1 in range(8):
                out_ap = ps0[n1][:] if n1 < 7 else ps0[n1][:, :, :]
                nc.tensor.matmul(
                    out_ap,
                    w1s[kk][:, :, n1 * 128:(n1 + 1) * 128],
                    xk0[kk][:, :, :],
                    start=(kk == 0), stop=(kk == 7), perf_mode=PM,
                )
        for n1 in range(8):
            kk2, i = n1 // 2, n1 % 2
            if i == 0:
                y1_t = y1pool.tile([128, 2, 512], FP8, tag="y1",
                                   name=f"y1b{kk2}")
                y1_slab0.append(y1_t)
            src_ap = ps0[n1][:] if n1 < 7 else ps0[n1][:, :, :]
            if n1 in CFG['bootact']:
                nc.scalar.activation(y1_slab0[kk2][:, i, :], src_ap, AF.Relu,
                                     bias=0.0, scale=1.0 / 32.0)
            else:
                nc.vector.tensor_scalar(y1_slab0[kk2][:, i, :], src_ap, 0.0,
                                        1.0 / 32.0, ALU.max, ALU.mult)

        def gram_tile(nm):
            if CFG['gram'] == 'gps':
                return gps.tile([128, 4, 128], F32, tag="g", name=nm)
            if CFG['gram'] == 'zg':
                gt = zgps.tile([128, 1024] if CFG['zg'] == 'pair'
                               else [128, 512], F32, tag="zg", name=nm)
                return gt
            return sps.tile([128, 512], F32, tag="sp", name=nm)

        def dma_x(s):
            xs = xpool.tile([128, 8, 2, 512], FP8, tag="xs", name=f"xs{s}")
            nc.sync.dma_start(xs[:], ins["xt"][:, s, :, :, :])
            return xs

        def dma_wc(a):
            wcs = wpool.tile([128, 4, JP], BF16, tag="wc", name=f"wc{a}")
            nc.sync.dma_start(wcs[:], ins["wcnt"][:, a, :, :])
            return wcs

        def emit_l1(s, xs, pads=None):
            y1 = []
            for n1 in range(8):
                kk2, i = n1 // 2, n1 % 2
                if i == 0:
                    y1_t = y1pool.tile([128, 2, 512], FP8, tag="y1",
                                       name=f"y1_{s}_{kk2}")
                    y1.append(y1_t)
                ps = l1ps.tile([128, 4, 128], F32, tag="l1")
                for kk in range(8):
                    nc.tensor.matmul(
                        ps[:, :, :],
                        w1s[kk][:, :, n1 * 128:(n1 + 1) * 128],
                        xs[:, kk, :, :],
                        start=(kk == 0), stop=(kk == 7), perf_mode=PM,
                    )
                _emit_relu(nc, y1[kk2][:, i, :], ps[:], n1)
                if pads and n1 in pads:
                    pads[n1]()
            return y1

        def emit_l2(s, y1, tpool):
            # zg^T [256p, 512b] in one 2-bank psum tile, cast to fp8 pair.
            # kk2-outer with chunk-interleave: the first matmuls only need
            # the slab's first relus, so PE rarely stalls on the last relu.
            zt = tpool.tile([128, 2, 512], FP8, tag="zt", name=f"zt{s}")
            if CFG['zg'] == 'pair':
                ps = zgps.tile([128, 1024], F32, tag="zg", name=f"zg{s}")
                pss = [ps[:, 0:512], ps[:, 512:1024]]
            else:
                pss = [zgps.tile([128, 512], F32, tag="zg",
                                 name=f"zg{s}_{c}")[:] for c in range(2)]
            for kk2 in range(4):
                for c in range(2):
                    nc.tensor.matmul(
                        pss[c],
                        w2s[:, kk2, :, c * 128:(c + 1) * 128],
                        y1[kk2][:, :, :],
                        start=(kk2 == 0), stop=(kk2 == 3), perf_mode=PM,
                    )
            nc.scalar.activation(zt[:, 0, :], pss[0], AF.Copy)
            if CFG['cast'] == 'act':
                nc.scalar.activation(zt[:, 1, :], pss[1], AF.Copy)
            else:
                nc.vector.tensor_copy(zt[:, 1, :], pss[1])
            return zt

        def _emit_diag4(gt, out_cols):
            # 4 diagonals at once: mask the packed grams with a block
            # identity, then one 3D tensor_reduce over the inner 128
            gd = dpool.tile([128, 4, 128], F32, tag="gd")
            nc.vector.tensor_tensor(gd[:], gt[:, :, :], identb[:], ALU.mult)
            nc.vector.tensor_reduce(out_cols, gd[:], mybir.AxisListType.X,
                                    ALU.add)

        def emit_anchor_post(a, anchT):
            # nsq_a for the slab's 4 tiles -> batched rn2a = 2/sqrt(nsq)
            gt = gram_tile("gA")
            for tt in range(4):
                g = gt[:, tt, :]
                at = anchT[:, :, tt * 128:(tt + 1) * 128]
                nc.tensor.matmul(g, at, at, start=True, stop=True,
                                 perf_mode=PM)
            cols = slice(4 * a, 4 * a + 4)
            _emit_diag4(gt, nsqA[:, cols])
            lnt = smp.tile([128, 4], F32, tag="lnt")
            nc.scalar.activation(lnt[:], nsqA[:, cols], AF.Ln)
            nc.scalar.activation(rn2a[:, cols], lnt[:], AF.Exp,
                                 bias=ln2c[:, 0:1], scale=-0.5)

        def emit_neg_sims(a, anchT, tts, edict):
            # S matmuls (PE) + exp (ACT) only; reduction emitted separately
            for tt in tts:
                t = 4 * a + tt
                at = anchT[:, :, tt * 128:(tt + 1) * 128]
                E = epool.tile([128, JP], BF16, tag="E", name=f"E{t}")
                for jc in range(4):
                    ps = sps.tile([128, 512], F32, tag="sp")
                    nc.tensor.matmul(
                        ps[:], at,
                        negtp[:, :, jc * 512:(jc + 1) * 512],
                        start=True, stop=True, perf_mode=PM)
                    nc.scalar.activation(E[:, jc * 512:(jc + 1) * 512],
                                         ps[:], AF.Exp, bias=0.0,
                                         scale=rn2a[:, t:t + 1])
                edict[t] = E

        bstate = {}

        def emit_neg_reduce(a, wcs, tts, edict):
            # prodE = E*W (Pool/DVE), then row-reduce on DVE/ACT
            # (accumulator DVE ops crash this runtime, so only reduce_sum
            # and activation-accum are used)
            for tt in tts:
                t = 4 * a + tt
                E = edict.pop(t)
                nred = CFG['nred']
                if a == 3 and CFG['tailnred']:
                    nred = CFG['tailnred']
                mode = nred[tt % len(nred)]
                if mode != 't':
                    pe_t = dpool.tile([128, JP], BF16, tag="ed")
                    if tt < CFG['pmult']:
                        nc.gpsimd.tensor_tensor(pe_t[:], E[:], wcs[:, tt, :],
                                                ALU.mult)
                    else:
                        nc.vector.tensor_tensor(pe_t[:], E[:], wcs[:, tt, :],
                                                ALU.mult)
                if mode == 't':
                    # binary add-tree at DVE 2x rate; Pool takes one of the
                    # two level-0 multiply halves (its only PSUM-free work)
                    H = JP // 2
                    h1 = dpool.tile([128, H], BF16, tag="th1")
                    if mode == 'p':
                        nc.gpsimd.tensor_tensor(h1[:], E[:, 0:H],
                                                wcs[:, tt, 0:H], ALU.mult)
                    else:
                        nc.vector.tensor_tensor(h1[:], E[:, 0:H],
                                                wcs[:, tt, 0:H], ALU.mult)
                    h2 = dpool.tile([128, H], BF16, tag="th2")
                    if mode == 'u':
                        nc.vector.tensor_tensor(h2[:], E[:, H:JP],
                                                wcs[:, tt, H:JP], ALU.mult)
                    else:
                        nc.gpsimd.tensor_tensor(h2[:], E[:, H:JP],
                                                wcs[:, tt, H:JP], ALU.mult)
                    s1 = dpool.tile([128, H], BF16, tag="ts1")
                    nc.vector.tensor_tensor(s1[:], h1[:], h2[:], ALU.add)
                    s2 = smp.tile([128, 512], BF16, tag="ts2")
                    if mode == 'T':
                        nc.gpsimd.tensor_tensor(s2[:], s1[:, 0:512],
                                                s1[:, 512:1024], ALU.add)
                    else:
                        nc.vector.tensor_tensor(s2[:], s1[:, 0:512],
                                                s1[:, 512:1024], ALU.add)
                    s3 = smp.tile([128, 256], BF16, tag="ts3")
                    nc.vector.tensor_tensor(s3[:], s2[:, 0:256],
                                            s2[:, 256:512], ALU.add)
                    s4 = smp.tile([128, 128], BF16, tag="ts4")
                    nc.vector.tensor_tensor(s4[:], s3[:, 0:128],
                                            s3[:, 128:256], ALU.add)
                    nc.vector.reduce_sum(negsumS[:, t:t + 1], s4[:],
                                         axis=mybir.AxisListType.X)
                    continue
                if mode == 'd':
                    nc.vector.reduce_sum(negsumS[:, t:t + 1], pe_t[:],
                                         axis=mybir.AxisListType.X)
                elif mode == 'a':
                    ed = dpool.tile([128, JP], BF16, tag="ed2")
                    nc.scalar.activation(ed[:], pe_t[:], AF.Copy, bias=0.0,
                                         scale=1.0,
                                         accum_out=negsumS[:, t:t + 1])
                else:  # split halves across DVE and ACT
                    h1 = smp.tile([128, 1], F32, tag="h1")
                    nc.vector.reduce_sum(h1[:], pe_t[:, 0:JP // 2],
                                         axis=mybir.AxisListType.X)
                    ed = dpool.tile([128, JP // 2], BF16, tag="ed2")
                    h2 = smp.tile([128, 1], F32, tag="h2")
                    nc.scalar.activation(ed[:], pe_t[:, JP // 2:JP], AF.Copy,
                                         bias=0.0, scale=1.0, accum_out=h2[:])
                    nc.vector.tensor_tensor(negsumS[:, t:t + 1], h1[:], h2[:],
                                            ALU.add)

        def emit_pos_post(k, anchT, posT):
            cols = slice(4 * k, 4 * k + 4)
            gtA = gram_tile("gPa")
            for tt in range(4):
                at = anchT[:, :, tt * 128:(tt + 1) * 128]
                pt = posT[:, :, tt * 128:(tt + 1) * 128]
                nc.tensor.matmul(gtA[:, tt, :], at, pt,
                                 start=True, stop=True, perf_mode=PM)
            _emit_diag4(gtA, prawT[:, cols])
            gtB = gram_tile("gPb")
            for tt in range(4):
                pt = posT[:, :, tt * 128:(tt + 1) * 128]
                nc.tensor.matmul(gtB[:, tt, :], pt, pt,
                                 start=True, stop=True, perf_mode=PM)
            _emit_diag4(gtB, nsqP[:, cols])
            lnt = smp.tile([128, 4], F32, tag="lnt")
            nc.scalar.activation(lnt[:], nsqP[:, cols], AF.Ln)
            nc.scalar.activation(rpmT[:, cols], lnt[:], AF.Exp, bias=0.0,
                                 scale=-0.5)
            # pos2 = praw * (2/||a||) * (1/||p||)
            pr = smp.tile([128, 4], F32, tag="pr")
            nc.vector.tensor_tensor(pr[:], prawT[:, cols], rn2a[:, cols],
                                    ALU.mult)
            nc.vector.tensor_tensor(pos2T[:, cols], pr[:], rpmT[:, cols],
                                    ALU.mult)

        def emit_loss(k):
            cols = slice(4 * k, 4 * k + 4)
            pe = smp.tile([128, 4], F32, tag="pe")
            nc.scalar.activation(pe[:], pos2T[:, cols], AF.Exp)
            tot = smp.tile([128, 4], F32, tag="tot")
            nc.vector.tensor_tensor(tot[:], pe[:], negsumS[:, cols], ALU.add)
            lse = smp.tile([128, 4], F32, tag="lse")
            nc.scalar.activation(lse[:], tot[:], AF.Ln)
            nc.vector.tensor_tensor(lossT[:, cols], lse[:], pos2T[:, cols],
                                    ALU.subtract)

        # ---- pipelined steady state (one-pair software pipeline) ----
        # pair k = (anchor slab 2k, positive slab 2k+1); negatives for pair
        # k-1 are interleaved into pair k so anchT/rn2a are long-ready and
        # the S-matmuls pad PE between L1 and L2 (hiding relu latency).
        xs1 = dma_x(1)
        nc.sync.dma_start(w2s[:], ins["w2"][:, :, :, :])
        nc.sync.dma_start(negtp[:], ins["negt"][:, :, :])
        nc.sync.dma_start(identb[:], ident_dram[:, :])
        wcs_tiles = {}
        xs_tiles = {1: xs1}

        anchT_prev = None
        anchT_cur = None
        edict = {}
        if CFG['negsched'] == 'split22':
            wcs_tiles[0] = dma_wc(0)
            for k in range(4):
                s_a, s_p = 2 * k, 2 * k + 1
                if k == 0:
                    y1A = y1_slab0
                else:
                    y1A = emit_l1(s_a, xs_tiles[s_a])
                if s_p + 1 < 8:
                    xs_tiles[s_p + 1] = dma_x(s_p + 1)
                if anchT_prev is not None:
                    emit_neg_sims(k - 1, anchT_prev, (2, 3), edict)
                anchT_cur = emit_l2(s_a, y1A, atpool)
                emit_anchor_post(k, anchT_cur)
                emit_neg_sims(k, anchT_cur, (0, 1), edict)
                if anchT_prev is not None:
                    emit_neg_reduce(k - 1, wcs_tiles[k - 1], (2, 3), edict)
                    emit_loss(k - 1)
                y1P = emit_l1(s_p, xs_tiles[s_p])
                if s_p + 2 < 8:
                    xs_tiles[s_p + 2] = dma_x(s_p + 2)
                if k + 1 < 4:
                    wcs_tiles[k + 1] = dma_wc(k + 1)
                emit_neg_reduce(k, wcs_tiles[k], (0, 1), edict)
                posT = emit_l2(s_p, y1P, ptpool)
                emit_pos_post(k, anchT_cur, posT)
                anchT_prev = anchT_cur
            emit_neg_sims(3, anchT_prev, (2, 3), edict)
            emit_neg_reduce(3, wcs_tiles[3], (2, 3), edict)
            emit_loss(3)
        else:
            anchT_cur = None
            for k in range(4):
                s_a, s_p = 2 * k, 2 * k + 1
                if k == 0:
                    y1A = y1_slab0
                else:
                    pads = None
                    if CFG['l1pad'] and anchT_prev is not None:
                        ap_, kk_ = anchT_prev, k - 1
                        pads = {3: (lambda a=ap_, q=kk_:
                                    emit_neg_sims(q, a, (0,), edict)),
                                5: (lambda a=ap_, q=kk_:
                                    emit_neg_sims(q, a, (1,), edict))}
                    y1A = emit_l1(s_a, xs_tiles[s_a], pads)
                if s_p + 1 < 8:
                    xs_tiles[s_p + 1] = dma_x(s_p + 1)
                if k == 0:
                    # issue wc0 after xs2: its 5.8us transfer must not
                    # delay the slab-2 activations (wc0 is consumed ~35us)
                    wcs_tiles[0] = dma_wc(0)
                p0i = CFG['p0inline']
                first_tts = (2,) if (p0i and k == 1) else (0, 1)
                late_tts = (3,) if (p0i and k == 1) else (2, 3)
                if anchT_prev is not None and not CFG['l1pad']:
                    emit_neg_sims(k - 1, anchT_prev, first_tts, edict)
                anchT_cur = emit_l2(s_a, y1A, atpool)
                if anchT_prev is not None:
                    emit_neg_sims(k - 1, anchT_prev, late_tts, edict)
                emit_anchor_post(k, anchT_cur)
                if p0i and k == 0:
                    emit_neg_sims(0, anchT_cur, (0, 1), edict)
                if anchT_prev is not None:
                    emit_neg_reduce(k - 1, wcs_tiles[k - 1],
                                    (2,) if (p0i and k == 1) else (0, 1),
                                    edict)
                if k == 3 and CFG['tail3'] == 'early':
                    emit_neg_sims(3, anchT_cur, (0, 1), edict)
                y1P = emit_l1(s_p, xs_tiles[s_p])
                if s_p + 2 < 8:
                    xs_tiles[s_p + 2] = dma_x(s_p + 2)
                if k + 1 < 4:
                    wcs_tiles[k + 1] = dma_wc(k + 1)
                if p0i and k == 0:
                    emit_neg_reduce(0, wcs_tiles[0], (0, 1), edict)
                if anchT_prev is not None:
                    emit_neg_reduce(k - 1, wcs_tiles[k - 1],
                                    (3,) if (p0i and k == 1) else (2, 3),
                                    edict)
                    emit_loss(k - 1)
                if k == 3 and CFG['tail3'] == 'early':
                    emit_neg_sims(3, anchT_cur, (2, 3), edict)
                    emit_neg_reduce(3, wcs_tiles[3], (0, 1), edict)
                posT = emit_l2(s_p, y1P, ptpool)
                if k == 3 and CFG['tailswap'] and CFG['tail3'] == 'late':
                    emit_neg_sims(3, anchT_cur, (0, 1, 2, 3), edict)
                    emit_pos_post(k, anchT_cur, posT)
                    emit_neg_reduce(3, wcs_tiles[3], (0, 1, 2, 3), edict)
                    emit_loss(3)
                    anchT_prev = anchT_cur
                    continue
                emit_pos_post(k, anchT_cur, posT)
                if k == 3:
                    # last pair: its own negatives inline (rn2a(3) is ready)
                    if CFG['tail3'] == 'early':
                        emit_neg_reduce(3, wcs_tiles[3], (2, 3), edict)
                    else:
                        emit_neg_sims(3, anchT_cur, (0, 1), edict)
                        emit_neg_reduce(3, wcs_tiles[3], (0, 1), edict)
                        emit_neg_sims(3, anchT_cur, (2, 3), edict)
                        emit_neg_reduce(3, wcs_tiles[3], (2, 3), edict)
                    emit_loss(3)
                anchT_prev = anchT_cur

        nc.sync.dma_start(out_losses[:, :], lossT[:])


def _emit_relu(nc, out_ap, ps_ap, n1):
    # y1 = relu(psum)/32; split ACT/DVE (GPSIMD cannot read PSUM), with the
    # last two groups of a slab draining on different engines concurrently
    if n1 in CFG['reluact']:
        nc.scalar.activation(out_ap, ps_ap, AF.Relu, bias=0.0,
                             scale=1.0 / 32.0)
    else:
        nc.vector.tensor_scalar(out_ap, ps_ap, 0.0, 1.0 / 32.0,
                                ALU.max, ALU.mult)


def _emit_general(tc, out_losses, ins, ident_dram):
    """General path (nonzero b1/beta/b2 or gamma != 1): bf16, sequential
    layout [A0..A3 P0..P3], full LayerNorm with explicit normalization."""
    from contextlib import ExitStack

    nc = tc.nc
    with ExitStack() as ctx:
        const = ctx.enter_context(tc.tile_pool(name="const", bufs=1))

        b1r = const.tile([128, 8], F32, tag="b1r")
        nc.sync.dma_start(b1r[:], ins["b1r"][:, :])
        b2r = const.tile([128, 256], F32, tag="b2r")
        nc.sync.dma_start(b2r[:], ins["b2r"][:, :])
        gr = const.tile([128, 256], F32, tag="gr")
        nc.sync.dma_start(gr[:], ins["gr"][:, :])
        br = const.tile([128, 256], F32, tag="br")
        nc.sync.dma_start(br[:], ins["br"][:, :])
        ident = const.tile([128, 128], BF16, tag="ident")
        nc.sync.dma_start(ident[:], ident_dram[:, :])

        negt = []
        for c in range(2):
            ngt = const.tile([128, JP], BF16, tag=f"negt{c}")
            negt.append(ngt)

        anchf = const.tile([128, BT * 256], F32, tag="anchf")
        anchT = []
        for c in range(2):
            at = const.tile([128, BL], BF16, tag=f"anchT{c}")
            anchT.append(at)
        pos2 = const.tile([128, BT], F32, tag="pos2")
        lossT = const.tile([128, BT], F32, tag="lossT")
        negsumT = const.tile([128, BT], F32, tag="negsumT")

        xpool = ctx.enter_context(tc.tile_pool(name="xk", bufs=32))
        y1pool = ctx.enter_context(tc.tile_pool(name="y1", bufs=16))
        smp = ctx.enter_context(tc.tile_pool(name="small", bufs=4))
        epool = ctx.enter_context(tc.tile_pool(name="ep", bufs=2))
        wpool = ctx.enter_context(tc.tile_pool(name="wp", bufs=2))

        w1s = []
        for k in range(16):
            w1t = const.tile([128, 1024], BF16, tag=f"w1s{k}")
            nc.sync.dma_start(w1t[:], ins["w1"][k * 128:(k + 1) * 128, :])
            w1s.append(w1t)
        w2s = []
        for k in range(8):
            w2t = const.tile([128, 256], BF16, tag=f"w2s{k}")
            nc.sync.dma_start(w2t[:], ins["w2"][k * 128:(k + 1) * 128, :])
            w2s.append(w2t)
        for c in range(2):
            nc.sync.dma_start(negt[c][:], ins["negt"][c * 128:(c + 1) * 128, :])

        l1ps = ctx.enter_context(tc.tile_pool(name="l1ps", bufs=3, space="PSUM"))
        l2ps = ctx.enter_context(tc.tile_pool(name="l2ps", bufs=2, space="PSUM"))
        sps = ctx.enter_context(tc.tile_pool(name="sps", bufs=2, space="PSUM"))
        tps = ctx.enter_context(tc.tile_pool(name="tps", bufs=1, space="PSUM"))

        def emit_slab(s):
            xk = []
            for k in range(16):
                xt_t = xpool.tile([128, 512], BF16, tag="xk")
                nc.sync.dma_start(
                    xt_t[:],
                    ins["xt"][k * 128:(k + 1) * 128, s * 512:(s + 1) * 512]
                )
                xk.append(xt_t)
            y1 = []
            for n1 in range(8):
                ps = l1ps.tile([128, 512], F32, tag="l1")
                for k in range(16):
                    nc.tensor.matmul(
                        ps[:],
                        w1s[k][:, n1 * 128:(n1 + 1) * 128],
                        xk[k][:],
                        start=(k == 0),
                        stop=(k == 15),
                    )
                y1_t = y1pool.tile([128, 512], BF16, tag="y1")
                nc.scalar.activation(
                    y1_t[:], ps[:], AF.Relu, bias=b1r[:, n1:n1 + 1], scale=1.0
                )
                y1.append(y1_t)

            for bsub in range(4):
                t = s * 4 + bsub
                ps2 = l2ps.tile([128, 256], F32, tag="l2")
                for k2 in range(8):
                    nc.tensor.matmul(
                        ps2[:],
                        y1[k2][:, bsub * 128:(bsub + 1) * 128],
                        w2s[k2][:],
                        start=(k2 == 0),
                        stop=(k2 == 7),
                    )
                y2 = smp.tile([128, 256], F32, tag="y2")
                nc.vector.tensor_tensor(y2[:], ps2[:], b2r[:], ALU.add)
                stats = smp.tile([128, 6], F32, tag="stats")
                nc.vector.bn_stats(stats[:], y2[:])
                aggr = smp.tile([128, 2], F32, tag="aggr")
                nc.vector.bn_aggr(aggr[:], stats[:])
                veps = smp.tile([128, 1], F32, tag="veps")
                nc.vector.tensor_scalar_add(veps[:], aggr[:, 1:2], 1e-5)
                std = smp.tile([128, 1], F32, tag="std")
                nc.scalar.activation(std[:], veps[:], AF.Sqrt, bias=0.0,
                                     scale=1.0)
                rstd = smp.tile([128, 1], F32, tag="rstd")
                nc.vector.reciprocal(rstd[:], std[:])
                xln = smp.tile([128, 256], F32, tag="xln")
                nc.vector.tensor_scalar(
                    xln[:], y2[:], aggr[:, 0:1], rstd[:], ALU.subtract,
                    ALU.mult)
                xg = smp.tile([128, 256], F32, tag="xg")
                nc.vector.tensor_tensor(xg[:], xln[:], gr[:], ALU.mult)
                xb = smp.tile([128, 256], F32, tag="xb")
                nc.vector.tensor_tensor(xb[:], xg[:], br[:], ALU.add)
                nsq = smp.tile([128, 1], F32, tag="nsq")
                dump = smp.tile([128, 256], F32, tag="dump")
                nc.scalar.activation(dump[:], xb[:], AF.Square, bias=0.0,
                                     scale=1.0, accum_out=nsq[:])
                nrm = smp.tile([128, 1], F32, tag="nrm")
                nc.scalar.activation(nrm[:], nsq[:], AF.Sqrt, bias=0.0,
                                     scale=1.0)
                nmx = smp.tile([128, 1], F32, tag="nmx")
                nc.vector.tensor_scalar_max(nmx[:], nrm[:], 1e-12)
                rn = smp.tile([128, 1], F32, tag="rn")
                nc.vector.reciprocal(rn[:], nmx[:])
                if t < BT:
                    nc.vector.tensor_scalar(
                        anchf[:, t * 256:(t + 1) * 256], xb[:], rn[:], None,
                        ALU.mult)
                    abf = smp.tile([128, 256], BF16, tag="abf")
                    nc.scalar.copy(abf[:], anchf[:, t * 256:(t + 1) * 256])
                    for c in range(2):
                        pst = tps.tile([128, 128], BF16, tag="tp")
                        nc.tensor.transpose(pst[:],
                                            abf[:, c * 128:(c + 1) * 128],
                                            ident[:])
                        nc.vector.tensor_copy(
                            anchT[c][:, t * 128:(t + 1) * 128], pst[:])
                else:
                    ta = t - BT
                    posf = smp.tile([128, 256], F32, tag="posf")
                    nc.vector.tensor_scalar(posf[:], xb[:], rn[:], None,
                                            ALU.mult)
                    prod = smp.tile([128, 256], F32, tag="prod")
                    nc.vector.tensor_tensor(
                        prod[:], posf[:], anchf[:, ta * 256:(ta + 1) * 256],
                        ALU.mult)
                    psim = smp.tile([128, 1], F32, tag="psim")
                    nc.vector.reduce_sum(psim[:], prod[:],
                                         axis=mybir.AxisListType.X)
                    nc.vector.tensor_scalar_mul(pos2[:, ta:ta + 1], psim[:],
                                                2.0)

        def emit_neg_tile(t):
            wct = wpool.tile([128, JP], BF16, tag="wc")
            nc.sync.dma_start(wct[:], ins["wcnt"][t * 128:(t + 1) * 128, :])
            E = epool.tile([128, JP], BF16, tag="E")
            for j in range(4):
                ps = sps.tile([128, 512], F32, tag="sp")
                for c in range(2):
                    nc.tensor.matmul(
                        ps[:],
                        anchT[c][:, t * 128:(t + 1) * 128],
                        negt[c][:, j * 512:(j + 1) * 512],
                        start=(c == 0),
                        stop=(c == 1),
                    )
                nc.scalar.activation(E[:, j * 512:(j + 1) * 512], ps[:],
                                     AF.Exp, bias=0.0, scale=2.0)
            prodE = epool.tile([128, JP], BF16, tag="prodE")
            nc.vector.tensor_tensor(prodE[:], E[:], wct[:], ALU.mult)
            nc.vector.reduce_sum(negsumT[:, t:t + 1], prodE[:],
                                 axis=mybir.AxisListType.X)

        for s in range(4):
            emit_slab(s)
        neg_sched = {4: range(0, 4), 5: range(4, 8), 6: range(8, 12),
                     7: range(12, 16)}
        for s in range(4, 8):
            emit_slab(s)
            for t in neg_sched[s]:
                emit_neg_tile(t)

        peT = smp.tile([128, BT], F32, tag="peT")
        nc.scalar.activation(peT[:], pos2[:], AF.Exp, bias=0.0, scale=1.0)
        totT = smp.tile([128, BT], F32, tag="totT")
        nc.vector.tensor_tensor(totT[:], peT[:], negsumT[:], ALU.add)
        lseT = smp.tile([128, BT], F32, tag="lseT")
        nc.scalar.activation(lseT[:], totT[:], AF.Ln, bias=0.0, scale=1.0)
        nc.vector.tensor_tensor(lossT[:], lseT[:], pos2[:], ALU.subtract)

        nc.sync.dma_start(out_losses[:, :], lossT[:])


def build_program(fast=True):
    key = (fast, V2)
    if key in _NC_CACHE:
        return _NC_CACHE[key]
    if fast:
        _ACT_KEEP[:] = ["natural_log_exp_and_others"]
    else:
        _ACT_KEEP[:] = ["natural_log_exp_and_others", "sqrt_and_others"]
    nc = bacc.Bacc("TRN2", target_bir_lowering=False, debug=False,
                   num_devices=NCORES)
    if fast:
        ins = {
            "xt": nc.dram_tensor("xt", [128, 8, 8, 2, 512], FP8,
                                 kind="ExternalInput").ap(),
            "w1": nc.dram_tensor("w1", [128, 8, 2, 1024], FP8,
                                 kind="ExternalInput").ap(),
            "w2": nc.dram_tensor("w2", [128, 4, 2, 256], FP8,
                                 kind="ExternalInput").ap(),
            "negt": nc.dram_tensor("negt", [128, 2, JP], FP8,
                                   kind="ExternalInput").ap(),
            "wcnt": nc.dram_tensor("wcnt", [128, 4, 4, JP], BF16,
                                   kind="ExternalInput").ap(),
        }
        out = nc.dram_tensor("losses", [128, BT], F32,
                             kind="ExternalOutput").ap()
        ident_dram = nc.inline_tensor(
            np.ascontiguousarray(
                np.broadcast_to(np.eye(128, dtype=np.float32),
                                (4, 128, 128)).transpose(1, 0, 2)),
            "identb").ap()
        with tile.TileContext(nc) as tc:
            if V2:
                _emit_fast2(tc, out, ins, ident_dram)
            else:
                _emit_fast(tc, out, ins, ident_dram)
    else:
        ins = {
            "xt": nc.dram_tensor("xt", [H, 2 * BL], BF16,
                                 kind="ExternalInput").ap(),
            "w1": nc.dram_tensor("w1", [H, HH], BF16,
                                 kind="ExternalInput").ap(),
            "w2": nc.dram_tensor("w2", [HH, P], BF16,
                                 kind="ExternalInput").ap(),
            "b1r": nc.dram_tensor("b1r", [128, 8], F32,
                                  kind="ExternalInput").ap(),
            "b2r": nc.dram_tensor("b2r", [128, 256], F32,
                                  kind="ExternalInput").ap(),
            "gr": nc.dram_tensor("gr", [128, 256], F32,
                                 kind="ExternalInput").ap(),
            "br": nc.dram_tensor("br", [128, 256], F32,
                                 kind="ExternalInput").ap(),
            "negt": nc.dram_tensor("negt", [P, JP], BF16,
                                   kind="ExternalInput").ap(),
            "wcnt": nc.dram_tensor("wcnt", [BL, JP], BF16,
                                   kind="ExternalInput").ap(),
        }
        out = nc.dram_tensor("losses", [128, BT], F32,
                             kind="ExternalOutput").ap()
        ident_dram = nc.inline_tensor(np.eye(128, dtype=BF), "ident").ap()
        with tile.TileContext(nc) as tc:
            _emit_general(tc, out, ins, ident_dram)
    nc.compile()
    _NC_CACHE[key] = nc
    return nc


def prepare_in_maps(hidden_states, positive_hidden, neg_buffer, W1, b1, W2, b2,
                    ln_gamma, ln_beta, neg_indices):
    hidden_states = np.asarray(hidden_states, dtype=np.float32)
    positive_hidden = np.asarray(positive_hidden, dtype=np.float32)
    neg_buffer = np.asarray(neg_buffer, dtype=np.float32)
    idx = np.asarray(neg_indices).astype(np.int64)

    g = np.asarray(ln_gamma, dtype=np.float32)
    beta = np.asarray(ln_beta, dtype=np.float32)
    b1a = np.asarray(b1, dtype=np.float32)
    b2a = np.asarray(b2, dtype=np.float32)
    W1a = np.asarray(W1, dtype=np.float32)
    W2a = np.asarray(W2, dtype=np.float32)
    # fast path: LayerNorm mean/rstd and all scalar scales cancel when the
    # affine pieces are identity and biases are zero.
    fast = bool(np.all(beta == 0.0) and np.all(g == 1.0)
                and np.all(b2a == 0.0) and np.all(b1a == 0.0))

    flat = (np.arange(B, dtype=np.int64)[:, None] * JP + idx).ravel()
    wcnt = np.bincount(flat, minlength=B * JP).reshape(B, JP).astype(np.float32)

    in_maps = []
    if fast:
        # W1 scaled into fp8 range; relu divides by 32
        w1q = np.ascontiguousarray(
            (W1a * W1S).astype(E4M3).reshape(8, 2, 128, 1024)
            .transpose(2, 0, 1, 3))                       # [128, kk, i, m]
        # W2 column-centered (folds LayerNorm mean) and scaled
        W2c = (W2a - W2a.mean(axis=1, keepdims=True)) * W2S
        w2q = np.ascontiguousarray(
            W2c.astype(E4M3).reshape(4, 2, 128, 256)
            .transpose(2, 0, 1, 3))                       # [128, kk2, i, pc]
        negq = np.zeros((128, 2, JP), E4M3)
        nb = neg_buffer.astype(E4M3)                      # [2000, 256]
        for c in range(2):
            negq[:, c, :NBUF] = nb[:, c * 128:(c + 1) * 128].T
        wcnt_b = wcnt.astype(BF)
        for cc in range(NCORES):
            rows = slice(cc * BL, (cc + 1) * BL)
            hs, ps = hidden_states[rows], positive_hidden[rows]
            if V2:
                # anchors-first slab order [A0..A3 P0..P3]
                xcat = np.concatenate([hs, ps], axis=0)   # [4096, 2048]
            else:
                blocks = []
                for sp in range(4):
                    blocks.append(hs[sp * 512:(sp + 1) * 512])
                    blocks.append(ps[sp * 512:(sp + 1) * 512])
                xcat = np.concatenate(blocks, axis=0)     # [4096, 2048]
            xq = xcat.astype(E4M3)
            # [128p, s, kk, i, n] <- xcat[s*512+n, kk*256+i*128+p]
            xr = np.ascontiguousarray(
                xq.reshape(8, 512, 8, 2, 128).transpose(4, 0, 2, 3, 1))
            wc = np.ascontiguousarray(
                wcnt_b[rows].reshape(4, 4, 128, JP).transpose(2, 0, 1, 3))
            in_maps.append({
                "xt": xr, "w1": w1q, "w2": w2q, "negt": negq, "wcnt": wc,
            })
        return in_maps, fast

    w1b = np.ascontiguousarray(W1a.astype(BF))
    w2b = np.ascontiguousarray(W2a.astype(BF))
    b1r = np.ascontiguousarray(b1a.reshape(8, 128).T)
    b2r = np.ascontiguousarray(np.tile(b2a, (128, 1)))
    gr = np.ascontiguousarray(np.tile(g, (128, 1)))
    br = np.ascontiguousarray(np.tile(beta, (128, 1)))
    negt = np.zeros((P, JP), BF)
    negt[:, :NBUF] = neg_buffer.astype(BF).T
    wcnt_b = wcnt.astype(BF)
    for cc in range(NCORES):
        rows = slice(cc * BL, (cc + 1) * BL)
        hs, ps = hidden_states[rows], positive_hidden[rows]
        xcat = np.concatenate([hs, ps], axis=0)
        xt_c = np.ascontiguousarray(xcat.astype(BF).T)  # [H, 2*BL]
        in_maps.append({
            "xt": xt_c, "w1": w1b, "w2": w2b, "b1r": b1r, "b2r": b2r,
            "gr": gr, "br": br, "negt": negt,
            "wcnt": np.ascontiguousarray(wcnt_b[rows]),
        })
    return in_maps, fast


def kernel(**inputs) -> np.ndarray:
    global LAST_RESULTS
    in_maps, fast = prepare_in_maps(**inputs)
    nc = build_program(fast)
    trace = bool(os.environ.get("BASS_TRACE_KERNEL"))
    res = run_bass_kernel_spmd(nc, in_maps, core_ids=list(range(NCORES)),
                               trace=trace)
    LAST_RESULTS = res
    total = np.float64(0.0)
    for c in range(NCORES):
        total += np.asarray(res.results[c]["losses"], dtype=np.float64).sum()
    return np.array(total / B, dtype=np.float32)

